# revision 35
# baseline (speedup 1.0000x reference)
"""Trainium2 Bass kernel for nn_ExaoneAttention (dense transformer attention).

Full-input contract: kernel(**inputs) takes the unsharded inputs and returns
the full [B, S, D] output. Internally shards across 8 NeuronCores:
2-way data parallel over batch x 4-way tensor parallel over kv heads
(2 kv heads = 8 query heads per core). Each core computes a partial
output through its Wo row-slice; the host sums the 4 partials per batch.

v2 design (vs the staged f32r baseline):
- fp16 operands everywhere (PE full rate + FWL weight-load hiding, which
  f32r disables; quantization noise ~2^-11 stays well inside the 2e-2 gate).
- Single fused pipeline per 512-query s-tile: QKV projection -> rope ->
  causal attention -> output projection, all SBUF-resident (no DRAM
  staging round trips). K/V accumulate into resident SBUF tiles; the Tile
  scheduler overlaps proj(st+1) matmuls into attention(st)'s exp stalls.
- V is projected directly in [seq, head_dim] orientation (hidden chunk as
  the stationary operand) so no PE transposes are needed.
- Causal masking is a multiplicative 0/1 fp16 mask applied after exp (2x
  DVE rate); softmax denominator accumulates in fp16 (<=16 adds, then an
  exact f32 ones-matmul partition reduce); reciprocal via the fast DVE
  approximation (~18 bits), broadcast back over partitions with a rank-1
  matmul.
"""

import contextlib
import ctypes
import os
import sys
import types

import numpy as np

# ---------------------------------------------------------------------------
# Problem constants (hardcoded per contract)
# ---------------------------------------------------------------------------
B, S, D = 2, 2048, 4096
H, HKV, HD = 32, 8, 128
G = H // HKV
THETA = 10000.0

NCORES = 8
BAT_SHARDS = 2
KV_SHARDS = 4
KVH = HKV // KV_SHARDS  # kv heads per core = 2
QH = KVH * G  # q heads per core = 8
DMC = D // 128  # 32 model-dim chunks
HALF = DMC // 2  # chunks per hidden slab

QT = 512  # query tile
NQT = S // QT  # 4
SC = 128  # key chunk
NSC = S // SC  # 16
DT = 512  # output d tile
NDT = D // DT  # 8

_SCALE = float(HD) ** -0.5


# ---------------------------------------------------------------------------
# Wait-count legalization: this walrus build rejects instructions carrying
# more than a small number of sync waits (fused fp32/fp32r matmul: >1;
# drain: >4). Hoist excess waits onto standalone NoOps on the same engine
# immediately before the offending instruction; AND-semantics are preserved
# by sequential same-engine execution.
# ---------------------------------------------------------------------------
def _legalize_waits(nc):
    import bass_rust
    import concourse.mybir as mybir

    counter = 0
    for f in nc.m.functions:
        for bb in f.blocks:
            il = bb.instructions
            i = 0
            while i < len(il):
                ins = il[i]
                si = ins.sync_info
                if si is None or len(si.on_wait) <= 1:
                    i += 1
                    continue
                waits = list(si.on_wait)
                pos = i
                for w in waits[1:]:
                    counter += 1
                    nop = mybir.InstNoOp(name=f"lgw-{counter}", ins=[], outs=[])
                    nop.engine = ins.engine
                    nop.sync_info = bass_rust.SyncInfo(on_wait=[w], on_update=[])
                    il.insert(pos, nop)
                    pos += 1
                    i += 1
                ins.sync_info = bass_rust.SyncInfo(
                    on_wait=waits[:1], on_update=list(si.on_update)
                )
                i += 1
    return counter


# ---------------------------------------------------------------------------
# Bass kernel builder (per-core program; same program on all 8 cores)
# ---------------------------------------------------------------------------
def _build_nc():
    import concourse.bass as bass
    import concourse.mybir as mybir
    from concourse.tile import TileContext

    f32 = mybir.dt.float32
    f16 = mybir.dt.float16
    AF = mybir.ActivationFunctionType

    nc = bass.Bass()

    # host-prearranged layouts (partition dim first everywhere)
    hi = nc.declare_dram_parameter("hi", [128, NQT, 2, HALF, QT], f16, isOutput=False)
    wq = nc.declare_dram_parameter("wq", [128, QH, DMC, 128], f16, isOutput=False)
    wk = nc.declare_dram_parameter("wk", [128, DMC, KVH * HD], f16, isOutput=False)
    wv = nc.declare_dram_parameter("wv", [128, DMC, KVH * HD], f16, isOutput=False)
    wo = nc.declare_dram_parameter("wo", [128, NDT, QH, DT], f16, isOutput=False)
    cc = nc.declare_dram_parameter("cc", [HD, S], f32, isOutput=False)
    ssn = nc.declare_dram_parameter("ssn", [HD, S], f32, isOutput=False)
    dmask = nc.declare_dram_parameter("dmask", [SC, G * QT], f16, isOutput=False)
    out = nc.declare_dram_parameter("out", [S, D], f16, isOutput=True)

    with TileContext(nc) as tc, contextlib.ExitStack() as top:
        singles = top.enter_context(tc.tile_pool(name="singles", bufs=1))
        hi_pool = top.enter_context(tc.tile_pool(name="hip", bufs=2))
        wq_pool = top.enter_context(tc.tile_pool(name="wqp", bufs=2))
        wo_pool = top.enter_context(tc.tile_pool(name="wop", bufs=2))
        qt_pool = top.enter_context(tc.tile_pool(name="qtp", bufs=2))
        ctx_pool = top.enter_context(tc.tile_pool(name="ctxp", bufs=2))
        rope_pool = top.enter_context(tc.tile_pool(name="ropep", bufs=2))
        pt_pool = top.enter_context(tc.tile_pool(name="ptp", bufs=6))
        acc_pool = top.enter_context(tc.tile_pool(name="accp", bufs=2))
        misc_pool = top.enter_context(tc.tile_pool(name="miscp", bufs=2))
        o_pool = top.enter_context(tc.tile_pool(name="op", bufs=4))
        ps_mm = top.enter_context(tc.tile_pool(name="ps_mm", bufs=2, space="PSUM"))
        ps_po = top.enter_context(tc.tile_pool(name="ps_po", bufs=2, space="PSUM"))
        ps_s = top.enter_context(tc.tile_pool(name="ps_s", bufs=2, space="PSUM"))
        ps_ctx = top.enter_context(tc.tile_pool(name="ps_ctx", bufs=1, space="PSUM"))
        ps_pb = top.enter_context(tc.tile_pool(name="ps_pb", bufs=1, space="PSUM"))

        # wk/wv are DMA'd inside the st=0 body, interleaved with the first
        # hidden slabs, so the first K-proj matmul waits only on the quarter
        # chunk ranges it actually reads (subtile deps) instead of 4.2MB of
        # weights queued ahead of the slabs
        wk_sb = singles.tile([128, DMC, KVH * HD], f16)
        wv_sb = singles.tile([128, DMC, KVH * HD], f16)
        # cc/ssn are DMA'd per s-tile slice inside the loop (keeps the first
        # projection matmuls off the critical path of these bulk loads)
        cc_sb = singles.tile([HD, S], f32)
        ssn_sb = singles.tile([HD, S], f32)
        # dmask is first read mid-attention(0); load it after the st=0 slab
        # DMAs are in flight rather than in the startup window
        dm_sb = singles.tile([SC, G * QT], f16)
        kT_sb = singles.tile([128, KVH, S], f16)
        v_sb = singles.tile([128, NSC, KVH, HD], f16)
        ones128 = singles.tile([128, 128], f16)
        nc.vector.memset(ones128, 1.0)
        nbias = singles.tile([128, 1], f32)
        nc.vector.memset(nbias, -4.0)

        # ~4us of dummy matmuls in the DMA-gated startup window: flips the
        # PE HAM clock gate to 8/8 (2.4GHz) before the first real matmul,
        # which otherwise runs its first ~16 MMs at the 1.2GHz cold clock.
        # Kept short: they serialize on one PSUM bank (~320ns each) and must
        # finish before the first slab/wk quarters land (~14us).
        for _ in range(12):
            pw = ps_pb.tile([128, 128], f32, name="pb")
            nc.tensor.matmul(pw, ones128, ones128, start=True, stop=True)

        def rope(dst, psum, ssl):
            """dst(f16) = neox-rope(psum) using cc and sign-folded ssn."""
            t1 = rope_pool.tile([HD, QT], f32, name="t1")
            t2 = rope_pool.tile([HD, QT], f32, name="t2")
            nc.vector.tensor_mul(t1, psum, cc_sb[:, ssl])
            nc.vector.tensor_mul(t2[:64], psum[64:], ssn_sb[:64, ssl])
            nc.vector.tensor_mul(t2[64:], psum[:64], ssn_sb[64:, ssl])
            nc.vector.tensor_add(dst, t1, t2)

        def emit_D(dst_st, dctx):
            """Output projection for s-tile dst_st from its ctx tile."""
            for dt in range(NDT):
                wot = wo_pool.tile([128, QH, DT], f16, name="wot")
                nc.sync.dma_start(out=wot, in_=wo[:, dt])
                for blk in range(QT // SC):
                    po = ps_po.tile([SC, DT], f32, name="po")
                    for h in range(QH):
                        nc.tensor.matmul(
                            po,
                            dctx[:, h, blk * SC : (blk + 1) * SC],
                            wot[:, h, :],
                            start=(h == 0),
                            stop=(h == QH - 1),
                        )
                    osb = o_pool.tile([SC, DT], f16, name="osb")
                    nc.scalar.copy(osb, po)
                    r0 = dst_st * QT + blk * SC
                    nc.sync.dma_start(
                        out=out[r0 : r0 + SC, dt * DT : (dt + 1) * DT], in_=osb
                    )

        prev_ctx = None
        for st in range(NQT):
            ssl = slice(st * QT, (st + 1) * QT)

            slabs = []
            for hh in range(2):
                slab = hi_pool.tile([128, HALF, QT], f16, name="slab")
                for qtr in range(4):
                    csl = slice(qtr * (HALF // 4), (qtr + 1) * (HALF // 4))
                    nc.sync.dma_start(out=slab[:, csl], in_=hi[:, st, hh, csl])
                    if st == 0 and hh == 0:
                        wsl = slice(qtr * (DMC // 4), (qtr + 1) * (DMC // 4))
                        nc.sync.dma_start(out=wk_sb[:, wsl], in_=wk[:, wsl])
                slabs.append(slab)
            if st == 0:
                nc.sync.dma_start(out=wv_sb[:, :HALF], in_=wv[:, :HALF])
                nc.sync.dma_start(out=wv_sb[:, HALF:], in_=wv[:, HALF:])
            nc.sync.dma_start(out=cc_sb[:, ssl], in_=cc[:, ssl])
            nc.sync.dma_start(out=ssn_sb[:, ssl], in_=ssn[:, ssl])
            if st == 0:
                nc.sync.dma_start(out=dm_sb, in_=dmask[:, :])

            # ---- K projection (+rope) into resident kT_sb ----
            for kv in range(KVH):
                pk = ps_mm.tile([128, QT], f32, name="mm")
                for c in range(DMC):
                    nc.tensor.matmul(
                        pk,
                        wk_sb[:, c, kv * HD : (kv + 1) * HD],
                        slabs[c // HALF][:, c % HALF, :],
                        start=(c == 0),
                        stop=(c == DMC - 1),
                    )
                rope(kT_sb[:, kv, ssl], pk, ssl)

            # ---- V projection, direct [seq, kv*HD] orientation ----
            for blk in range(QT // SC):
                pv = ps_mm.tile([128, KVH * HD], f32, name="mm")
                for c in range(DMC):
                    nc.tensor.matmul(
                        pv,
                        slabs[c // HALF][:, c % HALF, blk * SC : (blk + 1) * SC],
                        wv_sb[:, c, :],
                        start=(c == 0),
                        stop=(c == DMC - 1),
                    )
                nc.scalar.copy(v_sb[:, st * (QT // SC) + blk, :, :], pv)

            # ---- Q projection (+rope), wq streamed per head ----
            qt_t = qt_pool.tile([128, QH, QT], f16, name="qt")
            for h in range(QH):
                wqh = wq_pool.tile([128, DMC, 128], f16, name="wqh")
                nc.sync.dma_start(out=wqh, in_=wq[:, h])
                pq = ps_mm.tile([128, QT], f32, name="mm")
                for c in range(DMC):
                    nc.tensor.matmul(
                        pq,
                        wqh[:, c, :],
                        slabs[c // HALF][:, c % HALF, :],
                        start=(c == 0),
                        stop=(c == DMC - 1),
                    )
                rope(qt_t[:, h, :], pq, ssl)

            # ---- attention for this query tile ----
            ctx_t = ctx_pool.tile([128, QH, QT], f16, name="ctx")
            nk = G * (st + 1)
            for h in range(QH):
                kv = h // G
                pctx = ps_ctx.tile([128, QT], f32, name="cx")
                acc = acc_pool.tile([SC, QT], f16, name="acc")
                for i in range(nk):
                    # diagonal chunk t covers keys [i*SC, i*SC+SC); queries
                    # below t*SC are fully masked there -> narrow all work to
                    # the live query range [q0, QT)
                    t = i - G * st
                    q0 = t * SC if t > 0 else 0
                    qsl = slice(q0, QT)
                    pss = ps_s.tile([SC, QT], f32, name="ss")
                    nc.tensor.matmul(
                        pss[:, qsl],
                        kT_sb[:, kv, i * SC : (i + 1) * SC],
                        qt_t[:, h, qsl],
                        start=True,
                        stop=True,
                    )
                    pt = pt_pool.tile([SC, QT], f16, name="pt")
                    # bias -4 keeps exp inside fp16 range for extreme score
                    # tails (overflow at s*scale > 15.1 instead of 11.1); the
                    # e^-4 factor cancels exactly in the softmax normalization.
                    nc.scalar.activation(
                        pt[:, qsl], pss[:, qsl], AF.Exp, scale=_SCALE, bias=nbias
                    )
                    if t >= 0:
                        nc.vector.tensor_mul(
                            pt[:, qsl], pt[:, qsl], dm_sb[:, t * QT + q0 : (t + 1) * QT]
                        )
                    if i == 0:
                        nc.vector.tensor_copy(acc, pt)
                    else:
                        nc.vector.tensor_add(acc[:, qsl], acc[:, qsl], pt[:, qsl])
                    nc.tensor.matmul(
                        pctx[:, qsl],
                        v_sb[:, i, kv, :],
                        pt[:, qsl],
                        start=(i == 0),
                        stop=(i == nk - 1),
                    )
                # all-ones stationary: every output row of pred128 is the
                # partition-sum of acc -> reduce AND broadcast in one full-rate
                # matmul. 1/x then via exp(-ln(x)) on ScalarE (~2 ULP each; the
                # DVE reciprocal is ~4us per call and the custom-DVE fast
                # variant does not encode on this walrus build).
                pred128 = ps_pb.tile([128, QT], f32, name="pb")
                nc.tensor.matmul(pred128, ones128, acc, start=True, stop=True)
                ltmp = misc_pool.tile([128, QT], f32, name="ltmp")
                nc.scalar.activation(ltmp, pred128, AF.Ln)
                bc = misc_pool.tile([128, QT], f32, name="bc")
                nc.scalar.activation(bc, ltmp, AF.Exp, scale=-1.0)
                nc.vector.tensor_mul(ctx_t[:, h, :], pctx, bc)

            # ---- output projection, deferred by one s-tile: D(st-1) is
            # guaranteed-ready tensor filler for attention(st)'s exp stalls
            # (D(st) would only become ready near the end of attention(st))
            if prev_ctx is not None:
                emit_D(st - 1, prev_ctx)
            prev_ctx = ctx_t

        emit_D(NQT - 1, prev_ctx)

    _legalize_waits(nc)
    return nc


_NC_CACHE = {}
_last_exec_ns = None


def _get_nc():
    if "nc" not in _NC_CACHE:
        _NC_CACHE["nc"] = _build_nc()
    return _NC_CACHE["nc"]


# ---------------------------------------------------------------------------
# Optional NTFF profiling hook (used by the local test harness via
# KERNEL_TRACE=1; grading path leaves it off)
# ---------------------------------------------------------------------------
def _install_ntff_hook(so_path="/opt/axon/libaxon_pjrt.so"):
    if "antenv.axon_hooks" in sys.modules:
        return
    try:
        lib = ctypes.CDLL(so_path)
    except OSError:
        lib = None
    if lib is None or not hasattr(lib, "axon_start_nrt_profile"):
        hook = None
    else:
        lib.axon_start_nrt_profile.argtypes = [
            ctypes.POINTER(ctypes.c_int64),
            ctypes.c_size_t,
        ]
        lib.axon_start_nrt_profile.restype = ctypes.c_int64
        lib.axon_stop_nrt_profile.argtypes = [ctypes.c_char_p]
        lib.axon_stop_nrt_profile.restype = ctypes.c_int64

        @contextlib.contextmanager
        def hook(output_dir, device_ids):
            import jax

            jax.devices()
            if device_ids:
                ids = (ctypes.c_int64 * len(device_ids))(*device_ids)
                rc = lib.axon_start_nrt_profile(ids, len(device_ids))
            else:
                rc = lib.axon_start_nrt_profile(None, 0)
            if rc != 0:
                raise RuntimeError(f"axon_start_nrt_profile rc={rc}")
            try:
                yield
            finally:
                n = lib.axon_stop_nrt_profile(str(output_dir).encode())
                print(f"ntff profile: {n} file(s) -> {output_dir}", file=sys.stderr)

    mod = types.ModuleType("antenv.axon_hooks")
    mod.get_axon_ntff_profile_hook = lambda: hook
    sys.modules["antenv.axon_hooks"] = mod


# ---------------------------------------------------------------------------
# Host entry point
# ---------------------------------------------------------------------------
def kernel(hidden_states, position_ids, attention_mask, Wq, Wk, Wv, Wo):
    global _last_exec_ns
    from concourse import bass_utils

    hidden_states = np.asarray(hidden_states, dtype=np.float32)
    position_ids = np.asarray(position_ids)
    attention_mask = np.asarray(attention_mask)
    Wq = np.asarray(Wq, dtype=np.float32)
    Wk = np.asarray(Wk, dtype=np.float32)
    Wv = np.asarray(Wv, dtype=np.float32)
    Wo = np.asarray(Wo, dtype=np.float32)

    if not np.all(np.asarray(attention_mask) > 0):
        # Spec guarantees an all-ones mask; fall back to a host reference
        # implementation for the general case rather than mis-computing.
        return _host_reference(
            hidden_states, position_ids, attention_mask, Wq, Wk, Wv, Wo
        )

    # rope tables per batch: cc = [cos; cos], ssn = [-sin; sin]  (f32 [HD, S])
    half = HD // 2
    inv_freq = 1.0 / (THETA ** (np.arange(0, half, dtype=np.float32) / half))
    ccs, ssns = [], []
    for b in range(B):
        freqs = position_ids[b].astype(np.float32)[:, None] * inv_freq[None, :]
        cosT = np.cos(freqs).T.astype(np.float32)  # [64, S]
        sinT = np.sin(freqs).T.astype(np.float32)
        ccs.append(np.ascontiguousarray(np.concatenate([cosT, cosT], axis=0)))
        ssns.append(np.ascontiguousarray(np.concatenate([-sinT, sinT], axis=0)))

    # multiplicative causal masks for the diagonal blocks: block t in [0, G):
    # dmask[kk, t*QT + qq] = 1 if qq >= t*SC + kk else 0
    kk = np.arange(SC)[:, None]
    qq = np.arange(QT)[None, :]
    dmask = np.concatenate(
        [
            np.where(qq >= t * SC + kk, 1.0, 0.0).astype(np.float16)
            for t in range(G)
        ],
        axis=1,
    )
    dmask = np.ascontiguousarray(dmask)

    # hidden: [p, st, half, c_local, x]  (d = c*128 + p, s = st*QT + x)
    his = []
    for b in range(B):
        hiT = hidden_states[b].T.astype(np.float16)  # [D, S]
        t = hiT.reshape(DMC, 128, NQT, QT).transpose(1, 2, 0, 3)  # [p, st, c, x]
        his.append(np.ascontiguousarray(t.reshape(128, NQT, 2, HALF, QT)))

    in_maps = []
    for c in range(NCORES):
        b = c // KV_SHARDS
        m = c % KV_SHARDS
        FQ = QH * HD
        FKV = KVH * HD
        wq_s = Wq[:, m * FQ : (m + 1) * FQ].astype(np.float16)
        wq_pre = np.ascontiguousarray(
            wq_s.reshape(DMC, 128, QH, HD).transpose(1, 2, 0, 3)
        )  # [p, h, c, x]
        wk_s = Wk[:, m * FKV : (m + 1) * FKV].astype(np.float16)
        wk_pre = np.ascontiguousarray(
            wk_s.reshape(DMC, 128, FKV).transpose(1, 0, 2)
        )  # [p, c, kv*HD]
        wv_s = Wv[:, m * FKV : (m + 1) * FKV].astype(np.float16)
        wv_pre = np.ascontiguousarray(
            wv_s.reshape(DMC, 128, FKV).transpose(1, 0, 2)
        )
        wo_s = Wo[m * FQ : (m + 1) * FQ, :].astype(np.float16)
        wo_pre = np.ascontiguousarray(
            wo_s.reshape(QH, 128, NDT, DT).transpose(1, 2, 0, 3)
        )  # [p, dt, h, x]
        in_maps.append(
            {
                "hi": his[b],
                "wq": wq_pre,
                "wk": wk_pre,
                "wv": wv_pre,
                "wo": wo_pre,
                "cc": ccs[b],
                "ssn": ssns[b],
                "dmask": dmask,
            }
        )

    nc = _get_nc()
    trace = os.environ.get("KERNEL_TRACE", "") == "1"
    if trace:
        _install_ntff_hook()
        bass_utils.upload_artifacts = lambda tmpdir: f"local:{tmpdir}"
    res = bass_utils.run_bass_kernel_spmd(
        nc, in_maps, list(range(NCORES)), trace=trace
    )
    _last_exec_ns = res.exec_time_ns

    out = np.zeros((B, S, D), dtype=np.float32)
    for c in range(NCORES):
        out[c // KV_SHARDS] += np.asarray(res.results[c]["out"], dtype=np.float32)
    return out


def _host_reference(hidden_states, position_ids, attention_mask, Wq, Wk, Wv, Wo):
    """Numpy fallback for inputs outside the spec's guarantees."""
    q = (hidden_states @ Wq).reshape(B, S, H, HD)
    k = (hidden_states @ Wk).reshape(B, S, HKV, HD)
    v = (hidden_states @ Wv).reshape(B, S, HKV, HD)

    half = HD // 2
    inv_freq = 1.0 / (THETA ** (np.arange(0, half, dtype=np.float32) / half))
    freqs = position_ids.astype(np.float32)[..., None] * inv_freq
    cos = np.cos(freqs)[:, :, None, :]
    sin = np.sin(freqs)[:, :, None, :]

    def rope(x):
        x1, x2 = x[..., :half], x[..., half:]
        return np.concatenate([x1 * cos - x2 * sin, x2 * cos + x1 * sin], axis=-1)

    q, k = rope(q), rope(k)
    qg = q.reshape(B, S, HKV, G, HD)
    scores = np.einsum("bqhgd,bkhd->bhgqk", qg, k) * (HD**-0.5)
    causal = np.tril(np.ones((S, S), bool))
    mask = causal[None, None, None] & (attention_mask[:, None, None, None, :] > 0)
    scores = np.where(mask, scores, np.finfo(np.float32).min)
    scores = scores - scores.max(axis=-1, keepdims=True)
    probs = np.exp(scores)
    probs = probs / probs.sum(axis=-1, keepdims=True)
    ctx = np.einsum("bhgqk,bkhd->bqhgd", probs, v).reshape(B, S, H * HD)
    return (ctx @ Wo).astype(np.float32)


# revision 36
# speedup vs baseline: 1.0104x; 1.0104x over previous
"""Trainium2 Bass kernel for nn_ExaoneAttention (dense transformer attention).

Full-input contract: kernel(**inputs) takes the unsharded inputs and returns
the full [B, S, D] output. Internally shards across 8 NeuronCores:
2-way data parallel over batch x 4-way tensor parallel over kv heads
(2 kv heads = 8 query heads per core). Each core computes a partial
output through its Wo row-slice; the host sums the 4 partials per batch.

v2 design (vs the staged f32r baseline):
- fp16 operands everywhere (PE full rate + FWL weight-load hiding, which
  f32r disables; quantization noise ~2^-11 stays well inside the 2e-2 gate).
- Single fused pipeline per 512-query s-tile: QKV projection -> rope ->
  causal attention -> output projection, all SBUF-resident (no DRAM
  staging round trips). K/V accumulate into resident SBUF tiles; the Tile
  scheduler overlaps proj(st+1) matmuls into attention(st)'s exp stalls.
- V is projected directly in [seq, head_dim] orientation (hidden chunk as
  the stationary operand) so no PE transposes are needed.
- Causal masking is a multiplicative 0/1 fp16 mask applied after exp (2x
  DVE rate); softmax denominator accumulates in fp16 (<=16 adds, then an
  exact f32 ones-matmul partition reduce); reciprocal via the fast DVE
  approximation (~18 bits), broadcast back over partitions with a rank-1
  matmul.
"""

import contextlib
import ctypes
import os
import sys
import types

import numpy as np

# ---------------------------------------------------------------------------
# Problem constants (hardcoded per contract)
# ---------------------------------------------------------------------------
B, S, D = 2, 2048, 4096
H, HKV, HD = 32, 8, 128
G = H // HKV
THETA = 10000.0

NCORES = 8
BAT_SHARDS = 2
KV_SHARDS = 4
KVH = HKV // KV_SHARDS  # kv heads per core = 2
QH = KVH * G  # q heads per core = 8
DMC = D // 128  # 32 model-dim chunks
HALF = DMC // 2  # chunks per hidden slab

QT = 512  # query tile
NQT = S // QT  # 4
SC = 128  # key chunk
NSC = S // SC  # 16
DT = 512  # output d tile
NDT = D // DT  # 8

_SCALE = float(HD) ** -0.5


# ---------------------------------------------------------------------------
# Wait-count legalization: this walrus build rejects instructions carrying
# more than a small number of sync waits (fused fp32/fp32r matmul: >1;
# drain: >4). Hoist excess waits onto standalone NoOps on the same engine
# immediately before the offending instruction; AND-semantics are preserved
# by sequential same-engine execution.
# ---------------------------------------------------------------------------
def _legalize_waits(nc):
    import bass_rust
    import concourse.mybir as mybir

    counter = 0
    for f in nc.m.functions:
        for bb in f.blocks:
            il = bb.instructions
            i = 0
            while i < len(il):
                ins = il[i]
                si = ins.sync_info
                if si is None or len(si.on_wait) <= 1:
                    i += 1
                    continue
                waits = list(si.on_wait)
                pos = i
                for w in waits[1:]:
                    counter += 1
                    nop = mybir.InstNoOp(name=f"lgw-{counter}", ins=[], outs=[])
                    nop.engine = ins.engine
                    nop.sync_info = bass_rust.SyncInfo(on_wait=[w], on_update=[])
                    il.insert(pos, nop)
                    pos += 1
                    i += 1
                ins.sync_info = bass_rust.SyncInfo(
                    on_wait=waits[:1], on_update=list(si.on_update)
                )
                i += 1
    return counter


# ---------------------------------------------------------------------------
# Bass kernel builder (per-core program; same program on all 8 cores)
# ---------------------------------------------------------------------------
def _build_nc():
    import concourse.bass as bass
    import concourse.mybir as mybir
    from concourse.tile import TileContext

    f32 = mybir.dt.float32
    f16 = mybir.dt.float16
    AF = mybir.ActivationFunctionType

    nc = bass.Bass()

    # host-prearranged layouts (partition dim first everywhere)
    hi = nc.declare_dram_parameter("hi", [128, NQT, 2, HALF, QT], f16, isOutput=False)
    wq = nc.declare_dram_parameter("wq", [128, QH, DMC, 128], f16, isOutput=False)
    wk = nc.declare_dram_parameter("wk", [128, DMC, KVH * HD], f16, isOutput=False)
    wv = nc.declare_dram_parameter("wv", [128, DMC, KVH * HD], f16, isOutput=False)
    wo = nc.declare_dram_parameter("wo", [128, NDT, QH, DT], f16, isOutput=False)
    cc = nc.declare_dram_parameter("cc", [HD, S], f32, isOutput=False)
    ssn = nc.declare_dram_parameter("ssn", [HD, S], f32, isOutput=False)
    dmask = nc.declare_dram_parameter("dmask", [SC, G * QT], f16, isOutput=False)
    out = nc.declare_dram_parameter("out", [S, D], f16, isOutput=True)

    with TileContext(nc) as tc, contextlib.ExitStack() as top:
        singles = top.enter_context(tc.tile_pool(name="singles", bufs=1))
        hi_pool = top.enter_context(tc.tile_pool(name="hip", bufs=2))
        wq_pool = top.enter_context(tc.tile_pool(name="wqp", bufs=2))
        wo_pool = top.enter_context(tc.tile_pool(name="wop", bufs=2))
        qt_pool = top.enter_context(tc.tile_pool(name="qtp", bufs=2))
        ctx_pool = top.enter_context(tc.tile_pool(name="ctxp", bufs=2))
        rope_pool = top.enter_context(tc.tile_pool(name="ropep", bufs=2))
        pt_pool = top.enter_context(tc.tile_pool(name="ptp", bufs=6))
        acc_pool = top.enter_context(tc.tile_pool(name="accp", bufs=2))
        misc_pool = top.enter_context(tc.tile_pool(name="miscp", bufs=2))
        o_pool = top.enter_context(tc.tile_pool(name="op", bufs=4))
        ps_mm = top.enter_context(tc.tile_pool(name="ps_mm", bufs=2, space="PSUM"))
        ps_po = top.enter_context(tc.tile_pool(name="ps_po", bufs=2, space="PSUM"))
        ps_s = top.enter_context(tc.tile_pool(name="ps_s", bufs=2, space="PSUM"))
        ps_ctx = top.enter_context(tc.tile_pool(name="ps_ctx", bufs=1, space="PSUM"))
        ps_pb = top.enter_context(tc.tile_pool(name="ps_pb", bufs=1, space="PSUM"))

        # wk/wv are DMA'd inside the st=0 body, interleaved with the first
        # hidden slabs, so the first K-proj matmul waits only on the quarter
        # chunk ranges it actually reads (subtile deps) instead of 4.2MB of
        # weights queued ahead of the slabs
        wk_sb = singles.tile([128, DMC, KVH * HD], f16)
        wv_sb = singles.tile([128, DMC, KVH * HD], f16)
        # cc/ssn are DMA'd per s-tile slice inside the loop (keeps the first
        # projection matmuls off the critical path of these bulk loads)
        cc_sb = singles.tile([HD, S], f32)
        ssn_sb = singles.tile([HD, S], f32)
        # dmask is first read mid-attention(0); load it after the st=0 slab
        # DMAs are in flight rather than in the startup window
        dm_sb = singles.tile([SC, G * QT], f16)
        kT_sb = singles.tile([128, KVH, S], f16)
        v_sb = singles.tile([128, NSC, KVH, HD], f16)
        ones128 = singles.tile([128, 128], f16)
        nc.vector.memset(ones128, 1.0)
        nbias = singles.tile([128, 1], f32)
        nc.vector.memset(nbias, -4.0)



        def rope(dst, psum, ssl):
            """dst(f16) = neox-rope(psum) using cc and sign-folded ssn."""
            t1 = rope_pool.tile([HD, QT], f32, name="t1")
            t2 = rope_pool.tile([HD, QT], f32, name="t2")
            nc.vector.tensor_mul(t1, psum, cc_sb[:, ssl])
            nc.vector.tensor_mul(t2[:64], psum[64:], ssn_sb[:64, ssl])
            nc.vector.tensor_mul(t2[64:], psum[:64], ssn_sb[64:, ssl])
            nc.vector.tensor_add(dst, t1, t2)

        def emit_D(dst_st, dctx):
            """Output projection for s-tile dst_st from its ctx tile."""
            for dt in range(NDT):
                wot = wo_pool.tile([128, QH, DT], f16, name="wot")
                nc.sync.dma_start(out=wot, in_=wo[:, dt])
                for blk in range(QT // SC):
                    po = ps_po.tile([SC, DT], f32, name="po")
                    for h in range(QH):
                        nc.tensor.matmul(
                            po,
                            dctx[:, h, blk * SC : (blk + 1) * SC],
                            wot[:, h, :],
                            start=(h == 0),
                            stop=(h == QH - 1),
                        )
                    osb = o_pool.tile([SC, DT], f16, name="osb")
                    nc.scalar.copy(osb, po)
                    r0 = dst_st * QT + blk * SC
                    nc.sync.dma_start(
                        out=out[r0 : r0 + SC, dt * DT : (dt + 1) * DT], in_=osb
                    )

        prev_ctx = None
        for st in range(NQT):
            ssl = slice(st * QT, (st + 1) * QT)

            slabs = []
            for hh in range(2):
                slab = hi_pool.tile([128, HALF, QT], f16, name="slab")
                for qtr in range(4):
                    csl = slice(qtr * (HALF // 4), (qtr + 1) * (HALF // 4))
                    nc.sync.dma_start(out=slab[:, csl], in_=hi[:, st, hh, csl])
                    if st == 0 and hh == 0:
                        wsl = slice(qtr * (DMC // 4), (qtr + 1) * (DMC // 4))
                        nc.sync.dma_start(out=wk_sb[:, wsl], in_=wk[:, wsl])
                slabs.append(slab)
            if st == 0:
                nc.sync.dma_start(out=wv_sb[:, :HALF], in_=wv[:, :HALF])
                nc.sync.dma_start(out=wv_sb[:, HALF:], in_=wv[:, HALF:])
            nc.sync.dma_start(out=cc_sb[:, ssl], in_=cc[:, ssl])
            nc.sync.dma_start(out=ssn_sb[:, ssl], in_=ssn[:, ssl])
            if st == 0:
                nc.sync.dma_start(out=dm_sb, in_=dmask[:, :])

            # ---- K projection (+rope) into resident kT_sb ----
            for kv in range(KVH):
                pk = ps_mm.tile([128, QT], f32, name="mm")
                for c in range(DMC):
                    nc.tensor.matmul(
                        pk,
                        wk_sb[:, c, kv * HD : (kv + 1) * HD],
                        slabs[c // HALF][:, c % HALF, :],
                        start=(c == 0),
                        stop=(c == DMC - 1),
                    )
                rope(kT_sb[:, kv, ssl], pk, ssl)

            # ---- V projection, direct [seq, kv*HD] orientation ----
            for blk in range(QT // SC):
                pv = ps_mm.tile([128, KVH * HD], f32, name="mm")
                for c in range(DMC):
                    nc.tensor.matmul(
                        pv,
                        slabs[c // HALF][:, c % HALF, blk * SC : (blk + 1) * SC],
                        wv_sb[:, c, :],
                        start=(c == 0),
                        stop=(c == DMC - 1),
                    )
                nc.scalar.copy(v_sb[:, st * (QT // SC) + blk, :, :], pv)

            # ---- Q projection (+rope), wq streamed per head ----
            qt_t = qt_pool.tile([128, QH, QT], f16, name="qt")
            for h in range(QH):
                wqh = wq_pool.tile([128, DMC, 128], f16, name="wqh")
                nc.sync.dma_start(out=wqh, in_=wq[:, h])
                pq = ps_mm.tile([128, QT], f32, name="mm")
                for c in range(DMC):
                    nc.tensor.matmul(
                        pq,
                        wqh[:, c, :],
                        slabs[c // HALF][:, c % HALF, :],
                        start=(c == 0),
                        stop=(c == DMC - 1),
                    )
                rope(qt_t[:, h, :], pq, ssl)

            # ---- attention for this query tile ----
            ctx_t = ctx_pool.tile([128, QH, QT], f16, name="ctx")
            nk = G * (st + 1)
            for h in range(QH):
                kv = h // G
                pctx = ps_ctx.tile([128, QT], f32, name="cx")
                acc = acc_pool.tile([SC, QT], f16, name="acc")
                for i in range(nk):
                    # diagonal chunk t covers keys [i*SC, i*SC+SC); queries
                    # below t*SC are fully masked there -> narrow all work to
                    # the live query range [q0, QT)
                    t = i - G * st
                    q0 = t * SC if t > 0 else 0
                    qsl = slice(q0, QT)
                    pss = ps_s.tile([SC, QT], f32, name="ss")
                    nc.tensor.matmul(
                        pss[:, qsl],
                        kT_sb[:, kv, i * SC : (i + 1) * SC],
                        qt_t[:, h, qsl],
                        start=True,
                        stop=True,
                    )
                    pt = pt_pool.tile([SC, QT], f16, name="pt")
                    # bias -4 keeps exp inside fp16 range for extreme score
                    # tails (overflow at s*scale > 15.1 instead of 11.1); the
                    # e^-4 factor cancels exactly in the softmax normalization.
                    nc.scalar.activation(
                        pt[:, qsl], pss[:, qsl], AF.Exp, scale=_SCALE, bias=nbias
                    )
                    if t >= 0:
                        nc.vector.tensor_mul(
                            pt[:, qsl], pt[:, qsl], dm_sb[:, t * QT + q0 : (t + 1) * QT]
                        )
                    if i == 0:
                        nc.vector.tensor_copy(acc, pt)
                    else:
                        nc.vector.tensor_add(acc[:, qsl], acc[:, qsl], pt[:, qsl])
                    nc.tensor.matmul(
                        pctx[:, qsl],
                        v_sb[:, i, kv, :],
                        pt[:, qsl],
                        start=(i == 0),
                        stop=(i == nk - 1),
                    )
                # all-ones stationary: every output row of pred128 is the
                # partition-sum of acc -> reduce AND broadcast in one full-rate
                # matmul. 1/x then via exp(-ln(x)) on ScalarE (~2 ULP each; the
                # DVE reciprocal is ~4us per call and the custom-DVE fast
                # variant does not encode on this walrus build).
                pred128 = ps_pb.tile([128, QT], f32, name="pb")
                nc.tensor.matmul(pred128, ones128, acc, start=True, stop=True)
                ltmp = misc_pool.tile([128, QT], f32, name="ltmp")
                nc.scalar.activation(ltmp, pred128, AF.Ln)
                bc = misc_pool.tile([128, QT], f32, name="bc")
                nc.scalar.activation(bc, ltmp, AF.Exp, scale=-1.0)
                nc.vector.tensor_mul(ctx_t[:, h, :], pctx, bc)

            # ---- output projection, deferred by one s-tile: D(st-1) is
            # guaranteed-ready tensor filler for attention(st)'s exp stalls
            # (D(st) would only become ready near the end of attention(st))
            if prev_ctx is not None:
                emit_D(st - 1, prev_ctx)
            prev_ctx = ctx_t

        emit_D(NQT - 1, prev_ctx)

    _legalize_waits(nc)
    return nc


_NC_CACHE = {}
_last_exec_ns = None


def _get_nc():
    if "nc" not in _NC_CACHE:
        _NC_CACHE["nc"] = _build_nc()
    return _NC_CACHE["nc"]


# ---------------------------------------------------------------------------
# Optional NTFF profiling hook (used by the local test harness via
# KERNEL_TRACE=1; grading path leaves it off)
# ---------------------------------------------------------------------------
def _install_ntff_hook(so_path="/opt/axon/libaxon_pjrt.so"):
    if "antenv.axon_hooks" in sys.modules:
        return
    try:
        lib = ctypes.CDLL(so_path)
    except OSError:
        lib = None
    if lib is None or not hasattr(lib, "axon_start_nrt_profile"):
        hook = None
    else:
        lib.axon_start_nrt_profile.argtypes = [
            ctypes.POINTER(ctypes.c_int64),
            ctypes.c_size_t,
        ]
        lib.axon_start_nrt_profile.restype = ctypes.c_int64
        lib.axon_stop_nrt_profile.argtypes = [ctypes.c_char_p]
        lib.axon_stop_nrt_profile.restype = ctypes.c_int64

        @contextlib.contextmanager
        def hook(output_dir, device_ids):
            import jax

            jax.devices()
            if device_ids:
                ids = (ctypes.c_int64 * len(device_ids))(*device_ids)
                rc = lib.axon_start_nrt_profile(ids, len(device_ids))
            else:
                rc = lib.axon_start_nrt_profile(None, 0)
            if rc != 0:
                raise RuntimeError(f"axon_start_nrt_profile rc={rc}")
            try:
                yield
            finally:
                n = lib.axon_stop_nrt_profile(str(output_dir).encode())
                print(f"ntff profile: {n} file(s) -> {output_dir}", file=sys.stderr)

    mod = types.ModuleType("antenv.axon_hooks")
    mod.get_axon_ntff_profile_hook = lambda: hook
    sys.modules["antenv.axon_hooks"] = mod


# ---------------------------------------------------------------------------
# Host entry point
# ---------------------------------------------------------------------------
def kernel(hidden_states, position_ids, attention_mask, Wq, Wk, Wv, Wo):
    global _last_exec_ns
    from concourse import bass_utils

    hidden_states = np.asarray(hidden_states, dtype=np.float32)
    position_ids = np.asarray(position_ids)
    attention_mask = np.asarray(attention_mask)
    Wq = np.asarray(Wq, dtype=np.float32)
    Wk = np.asarray(Wk, dtype=np.float32)
    Wv = np.asarray(Wv, dtype=np.float32)
    Wo = np.asarray(Wo, dtype=np.float32)

    if not np.all(np.asarray(attention_mask) > 0):
        # Spec guarantees an all-ones mask; fall back to a host reference
        # implementation for the general case rather than mis-computing.
        return _host_reference(
            hidden_states, position_ids, attention_mask, Wq, Wk, Wv, Wo
        )

    # rope tables per batch: cc = [cos; cos], ssn = [-sin; sin]  (f32 [HD, S])
    half = HD // 2
    inv_freq = 1.0 / (THETA ** (np.arange(0, half, dtype=np.float32) / half))
    ccs, ssns = [], []
    for b in range(B):
        freqs = position_ids[b].astype(np.float32)[:, None] * inv_freq[None, :]
        cosT = np.cos(freqs).T.astype(np.float32)  # [64, S]
        sinT = np.sin(freqs).T.astype(np.float32)
        ccs.append(np.ascontiguousarray(np.concatenate([cosT, cosT], axis=0)))
        ssns.append(np.ascontiguousarray(np.concatenate([-sinT, sinT], axis=0)))

    # multiplicative causal masks for the diagonal blocks: block t in [0, G):
    # dmask[kk, t*QT + qq] = 1 if qq >= t*SC + kk else 0
    kk = np.arange(SC)[:, None]
    qq = np.arange(QT)[None, :]
    dmask = np.concatenate(
        [
            np.where(qq >= t * SC + kk, 1.0, 0.0).astype(np.float16)
            for t in range(G)
        ],
        axis=1,
    )
    dmask = np.ascontiguousarray(dmask)

    # hidden: [p, st, half, c_local, x]  (d = c*128 + p, s = st*QT + x)
    his = []
    for b in range(B):
        hiT = hidden_states[b].T.astype(np.float16)  # [D, S]
        t = hiT.reshape(DMC, 128, NQT, QT).transpose(1, 2, 0, 3)  # [p, st, c, x]
        his.append(np.ascontiguousarray(t.reshape(128, NQT, 2, HALF, QT)))

    in_maps = []
    for c in range(NCORES):
        b = c // KV_SHARDS
        m = c % KV_SHARDS
        FQ = QH * HD
        FKV = KVH * HD
        wq_s = Wq[:, m * FQ : (m + 1) * FQ].astype(np.float16)
        wq_pre = np.ascontiguousarray(
            wq_s.reshape(DMC, 128, QH, HD).transpose(1, 2, 0, 3)
        )  # [p, h, c, x]
        wk_s = Wk[:, m * FKV : (m + 1) * FKV].astype(np.float16)
        wk_pre = np.ascontiguousarray(
            wk_s.reshape(DMC, 128, FKV).transpose(1, 0, 2)
        )  # [p, c, kv*HD]
        wv_s = Wv[:, m * FKV : (m + 1) * FKV].astype(np.float16)
        wv_pre = np.ascontiguousarray(
            wv_s.reshape(DMC, 128, FKV).transpose(1, 0, 2)
        )
        wo_s = Wo[m * FQ : (m + 1) * FQ, :].astype(np.float16)
        wo_pre = np.ascontiguousarray(
            wo_s.reshape(QH, 128, NDT, DT).transpose(1, 2, 0, 3)
        )  # [p, dt, h, x]
        in_maps.append(
            {
                "hi": his[b],
                "wq": wq_pre,
                "wk": wk_pre,
                "wv": wv_pre,
                "wo": wo_pre,
                "cc": ccs[b],
                "ssn": ssns[b],
                "dmask": dmask,
            }
        )

    nc = _get_nc()
    trace = os.environ.get("KERNEL_TRACE", "") == "1"
    if trace:
        _install_ntff_hook()
        bass_utils.upload_artifacts = lambda tmpdir: f"local:{tmpdir}"
    res = bass_utils.run_bass_kernel_spmd(
        nc, in_maps, list(range(NCORES)), trace=trace
    )
    _last_exec_ns = res.exec_time_ns

    out = np.zeros((B, S, D), dtype=np.float32)
    for c in range(NCORES):
        out[c // KV_SHARDS] += np.asarray(res.results[c]["out"], dtype=np.float32)
    return out


def _host_reference(hidden_states, position_ids, attention_mask, Wq, Wk, Wv, Wo):
    """Numpy fallback for inputs outside the spec's guarantees."""
    q = (hidden_states @ Wq).reshape(B, S, H, HD)
    k = (hidden_states @ Wk).reshape(B, S, HKV, HD)
    v = (hidden_states @ Wv).reshape(B, S, HKV, HD)

    half = HD // 2
    inv_freq = 1.0 / (THETA ** (np.arange(0, half, dtype=np.float32) / half))
    freqs = position_ids.astype(np.float32)[..., None] * inv_freq
    cos = np.cos(freqs)[:, :, None, :]
    sin = np.sin(freqs)[:, :, None, :]

    def rope(x):
        x1, x2 = x[..., :half], x[..., half:]
        return np.concatenate([x1 * cos - x2 * sin, x2 * cos + x1 * sin], axis=-1)

    q, k = rope(q), rope(k)
    qg = q.reshape(B, S, HKV, G, HD)
    scores = np.einsum("bqhgd,bkhd->bhgqk", qg, k) * (HD**-0.5)
    causal = np.tril(np.ones((S, S), bool))
    mask = causal[None, None, None] & (attention_mask[:, None, None, None, :] > 0)
    scores = np.where(mask, scores, np.finfo(np.float32).min)
    scores = scores - scores.max(axis=-1, keepdims=True)
    probs = np.exp(scores)
    probs = probs / probs.sum(axis=-1, keepdims=True)
    ctx = np.einsum("bhgqk,bkhd->bqhgd", probs, v).reshape(B, S, H * HD)
    return (ctx @ Wo).astype(np.float32)
